# revision 3
# baseline (speedup 1.0000x reference)
"""Conv7x7(SAME) + LIF scan kernel for Trainium2, 8 NeuronCores.

Strategy (v4):
- Shard W=512 across cores: core c owns output cols [64c, 64c+64), receives a
  70-col slab (3-col halo each side, zero-padded) over all 512 rows and all
  128 timesteps, H-padded to 518 rows, laid out [518, 128, 70] in DRAM as an
  fp16 hi/lo pair (xh, xl).
- Conv needs ~fp32 precision for the LIF spike cascade (2-term fp16 flips
  ~8k spikes vs the ~626 budget; 3-term is ~110), i.e. the three products
  wh@xh + wh@xl + wl@xh. v4 computes them as TWO matmul chains instead of
  three by stacking operands along the contraction dim:
    main chain:  stationary rows 64..127 = wh band, moving = xh (64-row
                 sections), contraction 64.
    corr chain:  stationary rows 0..63 = wh band over xl, rows 64..127 =
                 wl band over xh, moving = [xl; xh] stacked, contraction 128.
  Output geometry: 10 blocks of 52 output rows (each consuming 58 of a
  64-row section). Blocks 0-4 go to PE array columns 0-63 (psum partitions
  0-63), blocks 5-9 to columns 64-127. The two column groups are independent
  PE sub-arrays: their matmuls STREAM CONCURRENTLY (2x col tiling, verified
  on HW: 400 half-width MMs == 200 full-width MMs wall time). Per step:
  7 dx taps x {main, corr} x {colgroup A, B} = 28 MMs of N=320, wall ~=
  7 x 2 x 320 cycles ~= 1.9us vs 2.97us for the v2/v3 21-MM layout.
  main+corr accumulate into ONE psum bank [128, 320] per step, so the LIF
  j-update still reads a single psum region.
- LIF on DVE, state [128, 320] (tracking j = 0.1*i; 0.1 folded into bands):
    u = 0.9*v + j ; m = (u <= 1) [bf16 out] ; v = (u <= 1)*u [fused STT] ;
    j = 0.9*j + psum [skipped at t=T-1 where it is dead].
  Host computes z = 1 - m.
- Input preloads in chunks (ring of 6, small head chunks); masks batch 4
  steps per mega-DMA except the last group which flushes per-step.
"""
import numpy as np
import concourse.bacc as bacc
import concourse.mybir as mybir
import concourse.tile as tile
from concourse.bass_utils import run_bass_kernel_spmd

T, H, WD, KK, PAD = 128, 512, 512, 7, 3
NCORES = 8
COLS = WD // NCORES           # 64 output cols per core
KP = COLS + 2 * PAD           # 70 input cols per core
NB = 10                       # row blocks per step (5 per column group)
BS = 52                       # output rows per block (58 of 64-row section)
HP = 4 * 122 + 128            # 616 slab rows (kept from v2: host zeros >518)
CK = 16                       # max timesteps per input chunk (tile capacity)
CHUNKS = [2, 2, 4, 8] + [16] * 7          # sizes; small head to start PE early
CSTART = [sum(CHUNKS[:i]) for i in range(len(CHUNKS))]
RING = 6                      # resident chunk ring
GB = 4                        # timesteps per output mega-DMA
NG = T // GB
NF = 320                      # psum/state free dim = 5 blocks x 64 cols

_cached = None


def _build():
    global _cached
    if _cached is not None:
        return _cached

    f32 = mybir.dt.float32
    f16 = mybir.dt.float16
    bf16 = mybir.dt.bfloat16
    Alu = mybir.AluOpType

    nc = bacc.Bacc("TRN2", debug=False, num_devices=NCORES)
    xh_d = nc.dram_tensor("xh", (HP, T, KP), f16, kind="ExternalInput")
    xl_d = nc.dram_tensor("xl", (HP, T, KP), f16, kind="ExternalInput")
    bmain_d = nc.dram_tensor("bmain", (128, KK * 64), f16,
                             kind="ExternalInput")
    bcorr_d = nc.dram_tensor("bcorr", (128, KK * 64), f16,
                             kind="ExternalInput")
    ms_d = nc.dram_tensor("ms", (NG, 128, GB * NF), bf16,
                          kind="ExternalOutput")

    CW = CK * KP              # 1120 cols per block-section in a chunk tile

    with tile.TileContext(nc) as tc:
        with (
            tc.tile_pool(name="pool", bufs=1) as pool,
            tc.tile_pool(name="psum", bufs=1, space="PSUM") as psum,
        ):
            wu_t = pool.tile([128, NF], f16, name="wu")
            nc.vector.memset(wu_t[:], 0.0)
            bmain_t = pool.tile([128, KK * 64], f16, name="bmain")
            bcorr_t = pool.tile([128, KK * 64], f16, name="bcorr")
            nc.sync.dma_start(bmain_t[:], bmain_d.ap())
            nc.sync.dma_start(bcorr_t[:], bcorr_d.ap())

            xcc = [pool.tile([128, NB * CW], f16, name=f"xc{r}")
                   for r in range(RING)]
            u_t = pool.tile([128, NF], f32, name="u")
            v_t = pool.tile([128, NF], f32, name="v")
            j_t = pool.tile([128, NF], f32, name="j")
            nc.vector.memset(v_t[:], 0.0)
            nc.vector.memset(j_t[:], 0.0)

            mg = [pool.tile([128, GB * NF], bf16, name=f"mg{i}")
                  for i in range(2)]
            pss = [psum.tile([128, NF], f32, name=f"ps{i}")
                   for i in range(8)]

            in_eng = [nc.sync, nc.scalar]
            n_in = [0]

            def load_chunk(ci):
                r = ci % RING
                sz = CHUNKS[ci]
                t0 = CSTART[ci]
                for b in range(NB):
                    for p0, src in ((0, xl_d), (64, xh_d)):
                        eng = in_eng[n_in[0] % len(in_eng)]
                        n_in[0] += 1
                        eng.dma_start(
                            xcc[r][p0:p0 + 64, b * CW:b * CW + sz * KP]
                            .rearrange("p (t q) -> p t q", q=KP),
                            src.ap()[BS * b:BS * b + 64, t0:t0 + sz, :])

            for ci in range(3):
                load_chunk(ci)

            # PE pstate warmup: harmless self-matmuls while preload streams in
            # (zero wu tile, no DMA dependency, so the PE starts ASAP)
            for i in range(6):
                nc.tensor.matmul(pss[7][0:128, :], wu_t[:, 0:128],
                                 wu_t[:, :], start=True, stop=True)

            step2chunk = []
            for ci, sz in enumerate(CHUNKS):
                step2chunk += [(ci, tl) for tl in range(sz)]

            for t in range(T):
                ck, tl = step2chunk[t]
                if tl == 0 and ck + 3 < len(CHUNKS):
                    load_chunk(ck + 3)
                r = ck % RING
                mc_f = xcc[r][:, :].rearrange(
                    "p (b t q) -> p b t q", b=NB, t=CK)
                mc_h = xcc[r][64:128, :].rearrange(
                    "p (b t q) -> p b t q", b=NB, t=CK)
                ps = pss[t % 8]
                for dx in range(KK):
                    first, last = dx == 0, dx == KK - 1
                    for g in range(2):          # column group: blocks 5g..5g+4
                        nc.tensor.matmul(
                            ps[64 * g:64 * g + 64, :],
                            bmain_t[64:128, dx * 64:(dx + 1) * 64],
                            mc_h[:, 5 * g:5 * g + 5, tl:tl + 1, dx:dx + 64],
                            start=first, stop=False,
                        )
                    for g in range(2):
                        nc.tensor.matmul(
                            ps[64 * g:64 * g + 64, :],
                            bcorr_t[:, dx * 64:(dx + 1) * 64],
                            mc_f[:, 5 * g:5 * g + 5, tl:tl + 1, dx:dx + 64],
                            start=False, stop=last,
                        )

                g = t // GB
                msl = mg[g % 2][:, (t % GB) * NF:(t % GB + 1) * NF]
                nc.vector.scalar_tensor_tensor(
                    u_t[:, :], v_t[:, :], 0.9, j_t[:, :],
                    Alu.mult, Alu.add)
                nc.vector.tensor_scalar(
                    msl, u_t[:, :], 1.0, None, Alu.is_le)
                nc.vector.scalar_tensor_tensor(
                    v_t[:, :], u_t[:, :], 1.0, u_t[:, :],
                    Alu.is_le, Alu.mult)
                if t < T - 1:
                    nc.vector.scalar_tensor_tensor(
                        j_t[:, :], j_t[:, :], 0.9, ps[:, :],
                        Alu.mult, Alu.add)

                if g < NG - 1:
                    if t % GB == GB - 1:
                        nc.gpsimd.dma_start(ms_d.ap()[g], mg[g % 2][:, :])
                else:
                    # last group: flush each step as soon as its mask lands to
                    # shorten the post-compute DMA drain tail
                    eng = nc.gpsimd if t % 2 == 0 else nc.sync
                    eng.dma_start(
                        ms_d.ap()[g][:, (t % GB) * NF:(t % GB + 1) * NF],
                        msl)

    nc.compile()
    _cached = nc
    return nc


def _bands(W01):
    # wh band in rows 64..127 (over xh) for the main chain; the corr chain
    # gets wh over xl in rows 0..63 plus wl (= W01 - wh) over xh in 64..127.
    wh = W01.astype(np.float16)
    wl = (W01 - wh.astype(np.float32)).astype(np.float16)
    bmain = np.zeros((128, KK * 64), np.float16)
    bcorr = np.zeros((128, KK * 64), np.float16)
    m = np.arange(BS)
    for dx in range(KK):
        for dy in range(KK):
            bmain[64 + m + dy, dx * 64 + m] = wh[dy, dx]
            bcorr[m + dy, dx * 64 + m] = wh[dy, dx]
            bcorr[64 + m + dy, dx * 64 + m] = wl[dy, dx]
    return bmain, bcorr


def kernel(x, W):
    x = np.asarray(x, np.float32)
    nc = _build()

    W01 = (np.float32(0.1) * np.asarray(W, np.float32).reshape(KK, KK))
    bmain, bcorr = _bands(W01)

    in_maps = []
    for c in range(NCORES):
        slab = np.zeros((T, HP, KP), np.float32)
        lo = COLS * c - PAD
        s0, s1 = max(0, lo), min(WD, lo + KP)
        slab[:, PAD:PAD + H, s0 - lo:s0 - lo + s1 - s0] = x[:, 0, :, s0:s1]
        xs = np.ascontiguousarray(slab.transpose(1, 0, 2))   # [616, 128, 70]
        xh = xs.astype(np.float16)
        xl = (xs - xh.astype(np.float32)).astype(np.float16)
        in_maps.append({"xh": xh, "xl": xl, "bmain": bmain, "bcorr": bcorr})

    res = run_bass_kernel_spmd(nc, in_maps, core_ids=list(range(NCORES)))

    z = np.empty((T, H, WD), np.float32)
    for c in range(NCORES):
        ms = np.asarray(res.results[c]["ms"]).astype(np.float32)
        # [NG, 128, GB*NF] -> [t, block(=half*5+blk5), row, col]
        m6 = ms.reshape(NG, 2, 64, GB, 5, 64).transpose(0, 3, 1, 4, 2, 5)
        m7 = m6.reshape(T, NB, 64, 64)
        zc = z[:, :, COLS * c:COLS * (c + 1)]
        zc[:, 0:9 * BS, :] = m7[:, 0:9, 0:BS, :].reshape(T, 9 * BS, 64)
        zc[:, 9 * BS:H, :] = m7[:, 9, 0:H - 9 * BS, :]
    return (np.float32(1.0) - z).reshape(T, 1, H, WD)


# revision 4
# speedup vs baseline: 1.9007x; 1.9007x over previous
"""Conv7x7(SAME) + LIF scan kernel for Trainium2, 8 NeuronCores.

Strategy (v6 = v3 + head/tail trims):
- Shard W=512 across cores: core c owns output cols [64c, 64c+64), receives a
  70-col slab (3-col halo each side, zero-padded) over all 512 rows and all
  128 timesteps, H-padded to 518 rows, laid out [518, 128, 70] in DRAM.
- Conv: contract over H on the TensorEngine. Stationary = banded matrix
  B[k, m] = W'[k-m, dx] mapping 128 input rows -> 122 output rows; the 512
  output rows split into 5 blocks at stride 122 (last block 24 valid rows).
  All 5 blocks merge into ONE matmul per tap via a 3D moving AP
  [128, (block, 64)], psum [128, 320] (stationary padded to 128 cols so Fast
  Weight Load engages). 7 dx taps accumulate in PSUM.
- Precision: the LIF spike cascade needs ~fp32 conv precision (2-term fp16
  splits flip ~8k spikes vs the ~626 budget), so the conv runs as a 3-term
  fp16 hi/lo decomposition (wh@xh + wh@xl + wl@xh), ~1.5e-7 max abs error.
  21 matmuls/step of N=320; measured PE streams ~0.442 ns/col under load so
  ~2.97us/step, PE-bound with zero mid-stream stalls.
  (Tried and rejected: 64x64 col-tiled variants — concurrent col-group
  streams do work, but per-MM LDW/mode-switch overheads eat the gain.)
- LIF (DVE): u = 0.9v + j; m = (u<=1) [bf16, DMA'd]; v = (u<=1)*u [fused
  STT]; j = 0.9j + psum [skipped at t=T-1 where dead]. Host does z = 1-m.
- Input preloads in chunks (ring of 6; 1-step head chunks, xh blocks DMA'd
  before xl so the first matmul starts as early as possible); output masks
  batch 4 steps per DMA except the last 8 steps which flush per-step across
  3 queues to shorten the drain tail.
"""
import numpy as np
import concourse.bacc as bacc
import concourse.mybir as mybir
import concourse.tile as tile
from concourse.bass_utils import run_bass_kernel_spmd

T, H, WD, KK, PAD = 128, 512, 512, 7, 3
NCORES = 8
COLS = WD // NCORES           # 64 output cols per core
KP = COLS + 2 * PAD           # 70 input cols per core
NB = 5                        # row blocks per step
BS = 122                      # output rows per block (contract 128, 7-tap)
BW = 128                      # stationary band width incl. zero pad (FWL)
HP = 4 * BS + 128             # 616 slab rows: rows beyond 518 are host zeros
CK = 16                       # max timesteps per input chunk (tile capacity)
CHUNKS = [1, 1, 2, 4, 8] + [16] * 7       # sizes; tiny head chunks
CSTART = [sum(CHUNKS[:i]) for i in range(len(CHUNKS))]
RING = 6                      # resident chunk ring
GB = 4                        # timesteps per output mega-DMA
NG = T // GB
FLUSH = T - 8                 # per-step mask flush from here on

_cached = None


def _build():
    global _cached
    if _cached is not None:
        return _cached

    f32 = mybir.dt.float32
    f16 = mybir.dt.float16
    bf16 = mybir.dt.bfloat16
    Alu = mybir.AluOpType

    nc = bacc.Bacc("TRN2", debug=False, num_devices=NCORES)
    xh_d = nc.dram_tensor("xh", (HP, T, KP), f16, kind="ExternalInput")
    xl_d = nc.dram_tensor("xl", (HP, T, KP), f16, kind="ExternalInput")
    bmh_d = nc.dram_tensor("bmh", (128, KK * BW), f16, kind="ExternalInput")
    bml_d = nc.dram_tensor("bml", (128, KK * BW), f16, kind="ExternalInput")
    ms_d = nc.dram_tensor("ms", (NG, BS, GB * NB * 64), bf16,
                          kind="ExternalOutput")

    CW = CK * KP              # 1120 cols per block in a chunk tile

    with tile.TileContext(nc) as tc:
        with (
            tc.tile_pool(name="pool", bufs=1) as pool,
            tc.tile_pool(name="psum", bufs=1, space="PSUM") as psum,
        ):
            wu_t = pool.tile([128, NB * 64], f16, name="wu")
            nc.vector.memset(wu_t[:], 0.0)
            bmh_t = pool.tile([128, KK * BW], f16, name="bmh")
            bml_t = pool.tile([128, KK * BW], f16, name="bml")
            nc.sync.dma_start(bmh_t[:], bmh_d.ap())
            nc.scalar.dma_start(bml_t[:], bml_d.ap())

            xhc = [pool.tile([128, NB * CW], f16, name=f"xh{r}")
                   for r in range(RING)]
            xlc = [pool.tile([128, NB * CW], f16, name=f"xl{r}")
                   for r in range(RING)]
            u_t = pool.tile([128, NB * 64], f32, name="u")
            v_t = pool.tile([128, NB * 64], f32, name="v")
            j_t = pool.tile([128, NB * 64], f32, name="j")
            nc.vector.memset(v_t[:], 0.0)
            nc.vector.memset(j_t[:], 0.0)

            mg = [pool.tile([128, GB * NB * 64], bf16, name=f"mg{i}")
                  for i in range(2)]
            pss = [psum.tile([128, NB * 64], f32, name=f"ps{i}")
                   for i in range(8)]

            in_eng = [nc.sync, nc.scalar]
            n_in = [0]

            def load_chunk(ci):
                r = ci % RING
                sz = CHUNKS[ci]
                t0 = CSTART[ci]
                # xh blocks first: the first matmul of a step reads only xh,
                # so its chunk-ready dependency resolves sooner
                for dst, src in ((xhc[r], xh_d), (xlc[r], xl_d)):
                    for b in range(NB):
                        eng = in_eng[n_in[0] % len(in_eng)]
                        n_in[0] += 1
                        eng.dma_start(
                            dst[:, b * CW:b * CW + sz * KP]
                            .rearrange("p (t q) -> p t q", q=KP),
                            src.ap()[BS * b:BS * b + 128, t0:t0 + sz, :])

            for ci in range(3):
                load_chunk(ci)

            # PE pstate warmup: harmless self-matmuls while preload streams in
            # (zero wu tile, no DMA dependency, so the PE starts ASAP)
            for i in range(10):
                nc.tensor.matmul(pss[7][0:BS, :], wu_t[:, 0:BS],
                                 wu_t[:, :], start=True, stop=True)

            step2chunk = []
            for ci, sz in enumerate(CHUNKS):
                step2chunk += [(ci, tl) for tl in range(sz)]

            for t in range(T):
                ck, tl = step2chunk[t]
                if tl == 0 and ck + 3 < len(CHUNKS):
                    load_chunk(ck + 3)
                r = ck % RING
                mvh = xhc[r][:, :].rearrange(
                    "p (b t q) -> p b t q", b=NB, t=CK)
                mvl = xlc[r][:, :].rearrange(
                    "p (b t q) -> p b t q", b=NB, t=CK)
                ps = pss[t % 8]
                n = 0
                for dx in range(KK):
                    for bm_t, mv in ((bmh_t, mvh), (bmh_t, mvl),
                                     (bml_t, mvh)):
                        nc.tensor.matmul(
                            ps[:, :],
                            bm_t[:, dx * BW:(dx + 1) * BW],
                            mv[:, :, tl:tl + 1, dx:dx + 64],
                            start=(n == 0), stop=(n == 3 * KK - 1),
                        )
                        n += 1

                g = t // GB
                msl = mg[g % 2][0:BS, (t % GB) * 320:(t % GB + 1) * 320]
                nc.vector.scalar_tensor_tensor(
                    u_t[0:BS, :], v_t[0:BS, :], 0.9, j_t[0:BS, :],
                    Alu.mult, Alu.add)
                nc.vector.tensor_scalar(
                    msl, u_t[0:BS, :], 1.0, None, Alu.is_le)
                nc.vector.scalar_tensor_tensor(
                    v_t[0:BS, :], u_t[0:BS, :], 1.0, u_t[0:BS, :],
                    Alu.is_le, Alu.mult)
                if t < T - 1:
                    nc.vector.scalar_tensor_tensor(
                        j_t[0:BS, :], j_t[0:BS, :], 0.9, ps[0:BS, :],
                        Alu.mult, Alu.add)

                if t < FLUSH:
                    if t % GB == GB - 1:
                        nc.gpsimd.dma_start(ms_d.ap()[g], mg[g % 2][0:BS, :])
                else:
                    # tail: flush each step as soon as its mask lands,
                    # rotating 3 queues so no single queue backlogs
                    eng = (nc.gpsimd, nc.sync, nc.scalar)[t % 3]
                    eng.dma_start(
                        ms_d.ap()[g][:, (t % GB) * 320:(t % GB + 1) * 320],
                        msl)

    nc.compile()
    _cached = nc
    return nc


def _bands(Wq):
    bm = np.zeros((128, KK * BW), np.float32)
    m = np.arange(BS)
    for dx in range(KK):
        for dy in range(KK):
            bm[m + dy, dx * BW + m] = Wq[dy, dx]
    return bm.astype(np.float16)


def kernel(x, W):
    x = np.asarray(x, np.float32)
    nc = _build()

    W01 = (np.float32(0.1) * np.asarray(W, np.float32).reshape(KK, KK))
    wh = W01.astype(np.float16)
    wl = (W01 - wh.astype(np.float32)).astype(np.float16)
    bmh = _bands(wh.astype(np.float32))
    bml = _bands(wl.astype(np.float32))

    in_maps = []
    for c in range(NCORES):
        slab = np.zeros((T, HP, KP), np.float32)
        lo = COLS * c - PAD
        s0, s1 = max(0, lo), min(WD, lo + KP)
        slab[:, PAD:PAD + H, s0 - lo:s0 - lo + s1 - s0] = x[:, 0, :, s0:s1]
        xs = np.ascontiguousarray(slab.transpose(1, 0, 2))   # [616, 128, 70]
        xh = xs.astype(np.float16)
        xl = (xs - xh.astype(np.float32)).astype(np.float16)
        in_maps.append({"xh": xh, "xl": xl, "bmh": bmh, "bml": bml})

    res = run_bass_kernel_spmd(nc, in_maps, core_ids=list(range(NCORES)))

    z = np.empty((T, H, WD), np.float32)
    for c in range(NCORES):
        ms = np.asarray(res.results[c]["ms"]).astype(np.float32)
        # [NG, BS, GB*NB*64] -> [t, block, row, w]
        m4 = ms.reshape(NG, BS, GB, NB, 64).transpose(0, 2, 3, 1, 4)
        m4 = m4.reshape(T, NB, BS, 64)
        zc = z[:, :, COLS * c:COLS * (c + 1)]
        zc[:, 0:4 * BS, :] = m4[:, 0:4].reshape(T, 4 * BS, 64)
        zc[:, 4 * BS:H, :] = m4[:, 4, 0:H - 4 * BS, :]
    return (np.float32(1.0) - z).reshape(T, 1, H, WD)
